# revision 1
# baseline (speedup 1.0000x reference)
"""Trainium2 Bass kernel for nn_Brain (gnn_message_passing, N=20000, E=20M, B=4, S=8).

Math (faithful to the reference):
    a_0 = zeros(N); a_0[:1000] = x0
    total_u[j] = c0[j] + sum_{d=1..u-1} sum_{e in E_d} w_e * a_{u-d}[from_e]   (to_e = j)
    c0[j]      = sum_{delay-0 edges} w_e * a_0[from_e]     (constant across steps)
    a_u = tanh(total_u), u = 1..8;  output = a_8[-1000:]   (delays >= 8 never fire)

Device design (8 NeuronCores, full inputs in / full output out):
  * to-neurons sharded: NC c owns j in [c*JP, (c+1)*JP), JP = N/8.
  * Per (NC, delay d=1..7) a dense fp8e4m3 plane [NFB*128 from-rows x JP to-cols],
    w pre-scaled by 64 (else N(0,0.01) weights flush to fp8 subnormals); a bf16
    delay-0 plane covers only from < 1000 rows. From-row index = f + pad*(f//JP)
    so each core's 2560-row slice is 128-aligned.
  * Per step u: for d = 1..u-1, for each from-block: PE matmul, stationary =
    snapshot a_{u-d} block [128 x 4] (LDWEIGHTS = 4 cols, nearly free), moving =
    plane columns fp8, accumulating [4 x JP] in PSUM banks across (d, fb).
  * total = PSUM + c0 (DVE), a_u = tanh(total/64) (ACT), AllGather the [4 x 2560]
    slice (gpsimd collective), then PE-transpose [32 x 128] chunks of the gathered
    [32 x 8*2560] into the f-partitioned SBUF snapshot table via PSUM + DVE.
"""
import sys
sys.path.insert(0, '/opt/trn_rl_repo')
import numpy as np
import ml_dtypes

NC_COUNT = 8
WSCALE = 64.0

FULL_CFG = dict(n=20000, e_in=1000, b=4, steps=8, nbank=5, chunk_fb=16, nbuf=4)


def derive(cfg):
    c = dict(cfg)
    n, b, s = c['n'], c['b'], c['steps']
    jp = n // NC_COUNT                      # to-neurons per core (2500)
    jpad = ((jp + 127) // 128) * 128        # 2560
    c.update(
        jp=jp, jpad=jpad,
        lfb=jpad // 128,                    # local from-blocks per core (20)
        nfb=NC_COUNT * (jpad // 128),       # global from-blocks (160)
        fpad=NC_COUNT * jpad,               # padded from-rows (20480)
        nfb0=(c['e_in'] + 127) // 128,      # delay-0 from-blocks (8)
        bank_j=jp // c['nbank'],            # 500
        ntr=jpad // 128,                    # post-gather transpose chunks (20)
    )
    assert jp % c['nbank'] == 0 and c['bank_j'] <= 512
    return c


def _mybir():
    import concourse.mybir as mybir
    return mybir


# --------------------------------------------------------------------------
# Bass program
# --------------------------------------------------------------------------
def build_bass(cfg, reps=1):
    from concourse import bass
    mybir = _mybir()
    c = derive(cfg)
    n, b, S = c['n'], c['b'], c['steps']
    jp, jpad, lfb, nfb, nfb0 = c['jp'], c['jpad'], c['lfb'], c['nfb'], c['nfb0']
    nbank, bank_j, chunk_fb = c['nbank'], c['bank_j'], c['chunk_fb']
    NBUF = c.get('nbuf', 2)
    ntr = c['ntr']
    NSNAP = S - 1
    TGRP = 16
    ngrp = (ntr + TGRP - 1) // TGRP
    e_in = c['e_in']
    # step-S is only needed on the last e_in outputs: restrict to the bank
    # range covering [jp - e_in, jp)
    lastbank0 = (jp - e_in) // bank_j
    assert lastbank0 * bank_j == jp - e_in, 'e_in must align to bank grid'

    nc = bass.Bass(target_bir_lowering=False)

    planes = [nc.declare_dram_parameter(f'w{d}', [128, nfb * jp], mybir.dt.uint8,
                                        isOutput=False) for d in range(1, S)]
    w0_t = nc.declare_dram_parameter('w0', [128, nfb0 * jp], mybir.dt.float32,
                                     isOutput=False)
    a0_t = nc.declare_dram_parameter('a0', [128, nfb0 * b], mybir.dt.float32,
                                     isOutput=False)
    id_t = nc.declare_dram_parameter('ident', [32, 32], mybir.dt.bfloat16,
                                     isOutput=False)
    out_t = nc.declare_dram_parameter('a8', [b, e_in], mybir.dt.float32,
                                      isOutput=True)
    ag_in = nc.dram_tensor('ag_in', [b, jpad], mybir.dt.bfloat16)
    ag_out = nc.dram_tensor('ag_out', [NC_COUNT * b, jpad], mybir.dt.bfloat16)

    def chunks_of(total, ch):
        out, x = [], 0
        while x < total:
            out.append((x, min(ch, total - x)))
            x += ch
        return out

    fb_chunks = chunks_of(nfb, chunk_fb)
    w0_chunks = chunks_of(nfb0, max(1, chunk_fb // 4))

    # pass lists per step: step 1 = the c0/w0 pass; steps u>=2: d descending so
    # the d=1 pass (which needs the freshest snapshot) comes last.
    def step_passes(u):
        if u == 1:
            return [0]
        return list(range(u - 1, 0, -1))

    # chunk schedule for one rep: (d, fb0, ch, u, jlo, jhi)
    sched = []
    for u in range(1, S + 1):
        jwin = (lastbank0 * bank_j, jp) if u == S else (0, jp)
        for d in step_passes(u):
            ch_list = w0_chunks if d == 0 else fb_chunks
            for (f0, ch) in ch_list:
                sched.append((d, f0, ch, u, jwin[0], jwin[1]))
    n_chunks = len(sched)
    step_first_chunk = {}
    for i, e in enumerate(sched):
        step_first_chunk.setdefault(e[3], i)
    step_first_chunk[S + 1] = n_chunks

    rhs_elems = chunk_fb * jp

    from contextlib import ExitStack
    with ExitStack() as _es:
        init_sem = _es.enter_context(nc.semaphore('init_sem'))
        pln_sems = [_es.enter_context(nc.semaphore(f'pln{i}')) for i in range(NBUF)]
        free_sem = _es.enter_context(nc.semaphore('free_sem'))
        tot_sem = _es.enter_context(nc.semaphore('tot_sem'))
        act_sem = _es.enter_context(nc.semaphore('act_sem'))
        agd_sem = _es.enter_context(nc.semaphore('agd_sem'))
        cc_sem = _es.enter_context(nc.semaphore('cc_sem'))
        tr_sem = _es.enter_context(nc.semaphore('tr_sem'))
        cp_sem = _es.enter_context(nc.semaphore('cp_sem'))
        ms_sem = _es.enter_context(nc.semaphore('ms_sem'))
        fin_sem = _es.enter_context(nc.semaphore('fin_sem'))
        sb_rhs = [_es.enter_context(nc.sbuf_tensor(f'sb_rhs{i}', [128, rhs_elems], mybir.dt.uint8))
                  for i in range(NBUF)]
        sb_snap = _es.enter_context(nc.sbuf_tensor('sb_snap', [128, NSNAP * nfb * b], mybir.dt.bfloat16))
        sb_a0 = _es.enter_context(nc.sbuf_tensor('sb_a0', [128, nfb0 * b], mybir.dt.float32))
        sb_id = _es.enter_context(nc.sbuf_tensor('sb_id', [32, 32], mybir.dt.bfloat16))
        sb_c0 = _es.enter_context(nc.sbuf_tensor('sb_c0', [b, jp], mybir.dt.float32))
        sb_tot = _es.enter_context(nc.sbuf_tensor('sb_tot', [b, jp], mybir.dt.float32))
        sb_a = _es.enter_context(nc.sbuf_tensor('sb_a', [b, jpad], mybir.dt.bfloat16))
        sb_a32 = _es.enter_context(nc.sbuf_tensor('sb_a32', [b, e_in], mybir.dt.float32))
        sb_ag = _es.enter_context(nc.sbuf_tensor('sb_ag', [NC_COUNT * b, jpad], mybir.dt.bfloat16))
        ps = _es.enter_context(nc.psum_tensor('ps', [b, nbank * 512], mybir.dt.float32))
        psT = _es.enter_context(nc.psum_tensor('psT', [128, 2 * TGRP * NC_COUNT * b], mybir.dt.bfloat16))

        block = _es.enter_context(nc.Block())
        AFT = mybir.ActivationFunctionType
        ps3 = ps.ap().rearrange('p (k j) -> p k j', k=nbank)
        snap4 = sb_snap.ap().rearrange('p (s f b) -> p s f b', s=NSNAP, f=nfb)
        a03 = sb_a0.ap().rearrange('p (f b) -> p f b', f=nfb0)
        psT4 = psT.ap().rearrange('p (h t cb) -> p h t cb', h=2, t=TGRP)
        sb_ag3 = sb_ag.ap().rearrange('cb (k p) -> cb k p', p=128)

        # -------------------------------------------- sync: plane DMA stream
        @block.sync
        def _(s):
            s.dma_start(out=sb_a0[:, :], in_=a0_t[:, :]).then_inc(init_sem, 16)
            s.dma_start(out=sb_id[:, :], in_=id_t[:, :]).then_inc(init_sem, 16)
            gi = 0
            for rep in range(reps):
                for (d, f0, ch, u, jlo, jhi) in sched:
                    if gi >= NBUF:
                        s.wait_ge(free_sem, gi - NBUF + 1)
                    buf = sb_rhs[gi % NBUF]
                    jw = jhi - jlo
                    if d == 0:
                        src = w0_t.ap().rearrange('p (f j) -> p f j', f=nfb0)[
                            :, f0:f0 + ch, jlo:jhi]
                        dst = buf.ap().bitcast(mybir.dt.float32)[
                            :, 0:ch * jw].rearrange('p (c j) -> p c j', c=ch)
                    else:
                        src = planes[d - 1].ap().rearrange(
                            'p (f j) -> p f j', f=nfb)[:, f0:f0 + ch, jlo:jhi]
                        dst = buf[:, 0:ch * jw].rearrange('p (c j) -> p c j', c=ch)
                    s.dma_start(out=dst, in_=src).then_inc(pln_sems[gi % NBUF], 16)
                    gi += 1
                s.wait_ge(act_sem, (rep + 1) * (S + 1))
                s.dma_start(out=out_t[:, :], in_=sb_a32[:, :]).then_inc(fin_sem, 16)
            s.wait_ge(fin_sem, 16 * reps)

        # -------------------------------------------- tensor: matmuls + transposes
        @block.tensor
        def _(t):
            ci = 0
            gtr = 0

            for rep in range(reps):
                snap_base = rep * (S - 1) * ngrp
                for u in range(1, S + 1):
                    jwin0 = lastbank0 * bank_j if u == S else 0
                    bank0 = lastbank0 if u == S else 0
                    if u >= 2:
                        t.wait_ge(tot_sem, rep * S + u - 1)
                    elif rep > 0:
                        t.wait_ge(tot_sem, rep * S)
                    passes = step_passes(u)
                    cnt_per_pass = len(w0_chunks) if u == 1 else len(fb_chunks)
                    for pi, d in enumerate(passes):
                        if d >= 1:
                            # needs snapshot s = u - d  (groups ngrp per gather)
                            t.wait_ge(cp_sem, snap_base + ngrp * (u - d))
                        for k in range(cnt_per_pass):
                            dd, f0, ch, uu, jlo, jhi = sched[ci % n_chunks]
                            assert dd == d and uu == u
                            jw = jhi - jlo
                            if ci == 0:
                                t.wait_ge(init_sem, 32)
                            t.wait_ge(pln_sems[ci % NBUF], 16 * (ci // NBUF + 1))
                            buf = sb_rhs[ci % NBUF]
                            if d == 0:
                                rhs3 = buf.ap().bitcast(mybir.dt.float32)[
                                    :, 0:ch * jw].rearrange('p (c j) -> p c j', c=ch)
                            else:
                                rhs3 = buf.ap().bitcast(mybir.dt.float8e4)[
                                    :, 0:ch * jw].rearrange('p (c j) -> p c j', c=ch)
                            for cc in range(ch):
                                fb = f0 + cc
                                if d == 0:
                                    lhsT = a03[:, fb, :]
                                else:
                                    lhsT = snap4[:, u - d - 1, fb, :]
                                nbk = nbank - bank0
                                for bi in range(nbk):
                                    bank = bank0 + bi
                                    mm = t.matmul(
                                        ps3[:, bank, 0:bank_j], lhsT,
                                        rhs3[:, cc, bi * bank_j:(bi + 1) * bank_j],
                                        start=(pi == 0 and k == 0 and cc == 0),
                                        stop=(pi == len(passes) - 1 and
                                              k == cnt_per_pass - 1 and cc == ch - 1),
                                        skip_group_check=True)
                            mm.then_inc(free_sem, 1)
                            ci += 1
                    if u <= S - 1:
                        t.wait_ge(agd_sem, 32 * (rep * (S - 1) + u))
                        for g in range(ngrp):
                            if gtr >= 2:
                                t.wait_ge(cp_sem, gtr - 1)
                            half = gtr % 2
                            k0 = g * TGRP
                            kcnt = min(TGRP, ntr - k0)
                            for kk in range(kcnt):
                                mm = t.transpose(psT4[:, half, kk, :],
                                                 sb_ag3[:, k0 + kk, :], sb_id[:, :])
                            mm.then_inc(tr_sem, 1)
                            gtr += 1

        # -------------------------------------------- scalar: tanh
        @block.scalar
        def _(a):
            a.wait_ge(ms_sem, 1)
            for rep in range(reps):
                for u in range(1, S + 1):
                    a.wait_ge(tot_sem, rep * S + u)
                    if u == S:
                        if rep > 0:
                            a.wait_ge(fin_sem, 16 * rep)
                        src = sb_tot.ap()[:, jp - e_in:jp]
                        a.activation(sb_a[:, jp - e_in:jp], src, AFT.Tanh,
                                     scale=1.0 / WSCALE).then_inc(act_sem, 1)
                        a.activation(sb_a32[:, :], src, AFT.Tanh,
                                     scale=1.0 / WSCALE).then_inc(act_sem, 1)
                    else:
                        src = sb_c0.ap()[:, :] if u == 1 else sb_tot.ap()[:, :]
                        a.activation(sb_a[:, 0:jp], src, AFT.Tanh,
                                     scale=1.0 / WSCALE).then_inc(act_sem, 1)

        # -------------------------------------------- vector: totals + snap copies
        @block.vector
        def _(v):
            gcp = 0
            v.memset(sb_a[:, :], 0.0).then_inc(ms_sem, 1)
            for rep in range(reps):
                for u in range(1, S + 1):
                    v.wait_ge(free_sem, rep * n_chunks + step_first_chunk[u + 1])
                    bank0 = lastbank0 if u == S else 0
                    nbk = nbank - bank0
                    ps_v = ps3[:, bank0:nbank, 0:bank_j]
                    c0_v = sb_c0.ap().rearrange('p (k j) -> p k j', k=nbank)[
                        :, bank0:nbank, :]
                    if u == 1:
                        v.tensor_copy(sb_c0.ap().rearrange(
                            'p (k j) -> p k j', k=nbank), ps_v).then_inc(tot_sem, 1)
                    else:
                        tt = sb_tot.ap().rearrange('p (k j) -> p k j', k=nbank)[
                            :, bank0:nbank, :]
                        v.tensor_tensor(tt, ps_v, c0_v,
                                        mybir.AluOpType.add).then_inc(tot_sem, 1)
                    if u <= S - 1:
                        si = u - 1
                        for g in range(ngrp):
                            v.wait_ge(tr_sem, gcp + 1)
                            half = gcp % 2
                            k0 = g * TGRP
                            kcnt = min(TGRP, ntr - k0)
                            src = psT4[:, half, 0:kcnt, :].rearrange(
                                'p t (c b) -> p t c b', c=NC_COUNT)
                            dst = snap4[:, si, :, :].rearrange(
                                'p (c kl) b -> p kl c b', c=NC_COUNT)[
                                :, k0:k0 + kcnt, :, :]
                            v.tensor_copy(dst, src).then_inc(cp_sem, 1)
                            gcp += 1

        # -------------------------------------------- gpsimd: allgather chain
        @block.gpsimd
        def _(g):
            for rep in range(reps):
                for u in range(1, S):
                    gs = rep * (S - 1) + u
                    g.wait_ge(act_sem, rep * (S + 1) + u)
                    g.dma_start(out=ag_in[:, :], in_=sb_a[:, :]).then_inc(agd_sem, 16)
                    g.wait_ge(agd_sem, 32 * gs - 16)
                    g.collective_compute(
                        'AllGather', mybir.AluOpType.bypass,
                        replica_groups=[list(range(NC_COUNT))],
                        ins=[ag_in.ap().opt()], outs=[ag_out.ap().opt()],
                    ).then_inc(cc_sem, 1)
                    g.wait_ge(cc_sem, gs)
                    g.dma_start(out=sb_ag[:, :], in_=ag_out[:, :]).then_inc(agd_sem, 16)
                    g.wait_ge(agd_sem, 32 * gs)

    return nc, c

# --------------------------------------------------------------------------
# Host preprocessing
# --------------------------------------------------------------------------
def preprocess(inputs, cfg):
    c = derive(cfg)
    n, b, S = c['n'], c['b'], c['steps']
    jp, jpad, nfb, nfb0 = c['jp'], c['jpad'], c['nfb'], c['nfb0']
    e_in = c['e_in']

    x0 = np.asarray(inputs['input_data'], np.float32)         # [B, IN]
    fr = np.asarray(inputs['from_idx'], np.int64)
    to = np.asarray(inputs['to_idx'], np.int64)
    dl = np.asarray(inputs['delays'], np.int64)
    w = np.asarray(inputs['connection_weights'], np.float32)

    keep = dl < S
    fr, to, dl, w = fr[keep], to[keep], dl[keep], w[keep]
    # delay-0 edges from f >= e_in contribute 0 forever (a_0 is 0 there)
    keep0 = ~((dl == 0) & (fr >= e_in))
    fr, to, dl, w = fr[keep0], to[keep0], dl[keep0], w[keep0]

    core = to // jp
    jl = to - core * jp
    frow = fr + (jpad - jp) * (fr // jp)      # padded from-row (128-aligned blocks)

    in_maps = [dict() for _ in range(NC_COUNT)]
    for cc in range(NC_COUNT):
        for d in range(S):
            m = (core == cc) & (dl == d)
            if d == 0:
                rows = fr[m]                   # < e_in, no padding shift there
                plane = np.zeros(128 * nfb0 * jp, np.float32)
                np.add.at(plane, (rows % 128) * (nfb0 * jp) +
                          (rows // 128) * jp + jl[m], w[m] * WSCALE)
                in_maps[cc]['w0'] = plane.reshape(128, nfb0 * jp)
            else:
                plane = np.zeros(128 * nfb * jp, np.float32)
                np.add.at(plane, (frow[m] % 128) * (nfb * jp) +
                          (frow[m] // 128) * jp + jl[m], w[m] * WSCALE)
                in_maps[cc][f'w{d}'] = plane.reshape(128, nfb * jp).astype(
                    ml_dtypes.float8_e4m3).view(np.uint8)

    a0 = np.zeros((128, nfb0, b), np.float32)
    for fb in range(nfb0):
        lo, hi = fb * 128, min((fb + 1) * 128, e_in)
        if hi > lo:
            a0[0:hi - lo, fb, :] = x0[:, lo:hi].T
    ident = np.eye(32, dtype=ml_dtypes.bfloat16)
    for cc in range(NC_COUNT):
        in_maps[cc]['a0'] = a0.reshape(128, nfb0 * b)
        in_maps[cc]['ident'] = ident
    return in_maps


# --------------------------------------------------------------------------
# PJRT runner (self-contained)
# --------------------------------------------------------------------------
class Runner:
    def __init__(self, nc, n_cores=NC_COUNT):
        import jax
        from jax.sharding import Mesh, PartitionSpec
        from jax.experimental.shard_map import shard_map
        import concourse.mybir as mybir
        from concourse.bass2jax import (_bass_exec_p, install_neuronx_cc_hook,
                                        partition_id_tensor)
        install_neuronx_cc_hook()
        self.jax = jax
        self.n_cores = n_cores
        partition_name = nc.partition_id_tensor.name if nc.partition_id_tensor else None
        dbg_name = nc.dbg_addr.name if nc.dbg_addr is not None else None
        in_names, out_names, out_avals, zero_outs = [], [], [], []
        for alloc in nc.m.functions[0].allocations:
            if not isinstance(alloc, mybir.MemoryLocationSet):
                continue
            name = alloc.memorylocations[0].name
            if alloc.kind == 'ExternalInput':
                if name not in (partition_name, dbg_name):
                    in_names.append(name)
            elif alloc.kind == 'ExternalOutput':
                out_names.append(name)
                shape = tuple(alloc.tensor_shape)
                dtype = mybir.dt.np(alloc.dtype)
                out_avals.append(jax.core.ShapedArray(shape, dtype))
                zero_outs.append(np.zeros(shape, dtype))
        self.in_names, self.out_names = in_names, out_names
        self.out_avals, self.zero_outs = out_avals, zero_outs
        all_in = list(in_names) + list(out_names)
        if dbg_name is not None:
            all_in.append(dbg_name)
        if partition_name is not None:
            all_in.append(partition_name)
        has_dbg = dbg_name is not None

        def _body(*args):
            operands = list(args)
            if has_dbg:
                operands.append(jax.numpy.zeros((1, 2), jax.numpy.uint32))
            if partition_name is not None:
                operands.append(partition_id_tensor())
            return tuple(_bass_exec_p.bind(
                *operands, out_avals=tuple(out_avals), in_names=tuple(all_in),
                out_names=tuple(out_names), lowering_input_output_aliases=(),
                sim_require_finite=False, sim_require_nnan=False, nc=nc))

        devices = jax.devices()[:n_cores]
        mesh = Mesh(np.asarray(devices), ('core',))
        self._fn = jax.jit(
            shard_map(_body, mesh=mesh,
                      in_specs=(PartitionSpec('core'),) * (len(in_names) + len(out_names)),
                      out_specs=(PartitionSpec('core'),) * len(out_names),
                      check_rep=False),
            keep_unused=True)
        self._sharding = jax.sharding.NamedSharding(mesh, PartitionSpec('core'))

    def put_inputs(self, in_maps):
        jax = self.jax
        dev_in = [jax.device_put(
            np.concatenate([np.asarray(m[name]) for m in in_maps], axis=0),
            self._sharding) for name in self.in_names]
        dev_zero = [jax.device_put(
            np.zeros((self.n_cores * z.shape[0], *z.shape[1:]), z.dtype),
            self._sharding) for z in self.zero_outs]
        return dev_in, dev_zero

    def run(self, dev_in, dev_zero):
        outs = self._fn(*dev_in, *dev_zero)
        self.jax.block_until_ready(outs)
        return outs

    def results(self, outs):
        return [
            {name: np.asarray(outs[i]).reshape(self.n_cores, *self.out_avals[i].shape)[c]
             for i, name in enumerate(self.out_names)}
            for c in range(self.n_cores)
        ]


# --------------------------------------------------------------------------
# public entry point
# --------------------------------------------------------------------------
_CACHE = {}


def _get_runner(cfg_key):
    if cfg_key not in _CACHE:
        cfg = dict(FULL_CFG)
        nc, c = build_bass(cfg)
        _CACHE[cfg_key] = (Runner(nc), c)
    return _CACHE[cfg_key]


def kernel(input_data, from_idx, to_idx, delays, connection_weights, steps):
    assert int(steps) == FULL_CFG['steps']
    runner, c = _get_runner('full')
    in_maps = preprocess(
        dict(input_data=input_data, from_idx=from_idx, to_idx=to_idx,
             delays=delays, connection_weights=connection_weights), FULL_CFG)
    dev_in, dev_zero = runner.put_inputs(in_maps)
    outs = runner.run(dev_in, dev_zero)
    res = runner.results(outs)
    # a_8[-e_in:] lives in core 7's trailing e_in columns == its 'a8' output
    return res[NC_COUNT - 1]['a8'].astype(np.float32)

